# revision 31
# baseline (speedup 1.0000x reference)
"""Mixture-of-Experts (top-1 routing) Trainium2 kernel.

Strategy (expert-parallel with one overflow slot, per sharding hint):
 - Router (softmax / argmax / top-prob) evaluated on host — 8192x8, i.e.
   0.002% of the FLOPs; its cost is dispatch bookkeeping.
 - Core e owns expert e.  The first MT-1 m-tiles of a core hold tokens of
   its primary expert; the last m-tile is an overflow slot (own-expert
   overflow, or up to 128 tokens of one overloaded foreign expert, using
   the core's secondary weight tensor).
 - Each core runs a dense [C,1024] @ [1024,1024] GEMM on the TensorEngine
   with fp16 operands and fp32 PSUM accumulation (~4.5e-4 max rel err
   end-to-end).  PSUM eviction fuses the bias in a single DVE op per
   half-tile: out = (bias * top_p) + psum.

Pipeline layout (v7), engineered around three measured hardware facts:
 (a) The PE HAM clock gate opens (1.2 -> 2.4 GHz) at the first
     fully-busy free-running ~3.4 us activity window, and ANY PE idle
     gap before then (or a >3.4 us gap later) costs a half-clock
     window.  Warm-up matmuls bridge from the earliest possible
     instruction (~7.5 us) to the first real k-tile, and the input
     stream is paced so the PE never starves.
 (b) Each HWDGE ring drains its transfers near-serially at a rate that
     grows with the per-partition row size, with ~1.5 us first-transfer
     arm latency and ~0.5 us completion fan-in.  So: few transfers, in
     consumption order, host-packed contiguous; k0 and k1 ship solo for
     the earliest possible stream start, later k's in pairs.
 (c) The 16 SDMA engines round-robin all active queues, so a transfer's
     wire time stretches with whatever else is in flight: the SWDGE
     (GpSimd) side stream of scale/bias tiles is split and gated so
     each piece moves only after the ring transfers it would compete
     with have completed, and the bulky overflow weights (needed at
     ~75% of the stream) are queued behind the first output tiles.
 - Matmuls run in half-chunk passes: (4 m-tiles x 1 n-half) accumulated
   over all k.  A pass's 4 PSUM banks are evicted while the next pass
   runs on the other 4 banks, pipelining evictions and output DMAs
   behind the TensorEngine instead of bunching them at the tail.
 - Host scatters the compact per-core outputs back to token order
   (the "second all-to-all" / unshard step).
"""

import numpy as np

T, H, E = 8192, 1024, 8
N_CORES = 8
P = 128
KT = H // P          # 8 contraction tiles
NFREE = 512          # matmul moving free dim (one PSUM bank of fp32)
NT = H // NFREE      # 2 output column tiles
CH = 4               # m-tiles per half-chunk (4m x 1n = 4 PSUM banks)

KPAIRS = [[0, 1], [2, 3], [4, 5], [6, 7]]      # k-pair block transfers
HGROUPS = [[0, 1, 2, 3], [4, 5, 6, 7]]         # bulk 4-k groups
WARM = 12            # warm-up matmuls (384-col moving, ~320 ns cold each)

_BUILD_CACHE = {}


def _build(MT):
    """Build the SPMD Bass module for MT m-tiles per core (C = MT*128)."""
    import concourse.mybir as mybir
    import concourse.tile as tile
    from concourse import bacc

    C = MT * P
    A = min(CH, MT) * P  # xt columns consumed by the first chunk
    B = C - A            # remaining xt columns (chunks 1..)
    DT = mybir.dt.float16    # half-precision I/O, full-rate matmul
    F32 = mybir.dt.float32
    F16 = mybir.dt.float16
    ALU = mybir.AluOpType

    nc = bacc.Bacc("TRN2", target_bir_lowering=False, debug=False,
                   num_devices=N_CORES)

    # blk j = [xt_2j[:, :A] | xt_2j+1[:, :A] | w_2j[:, :NF] | w_2j+1[:, :NF]]
    BLKW = 2 * A + 2 * NFREE
    blk_d = [nc.dram_tensor(f"blk{j}", [P, BLKW], DT,
                            kind="ExternalInput").ap()
             for j in range(len(KPAIRS))]
    wb_d = [nc.dram_tensor(f"wb{gi}", [P, len(g) * NFREE], DT,
                           kind="ExternalInput").ap()
            for gi, g in enumerate(HGROUPS)]
    w2_d = [nc.dram_tensor(f"w2_{gi}", [P, len(g) * H], DT,
                           kind="ExternalInput").ap()
            for gi, g in enumerate(HGROUPS)]
    biasa_d = nc.dram_tensor("biasa", [P, NFREE], F16, kind="ExternalInput").ap()
    biasb_d = nc.dram_tensor("biasb", [P, NFREE], F16, kind="ExternalInput").ap()
    bias2_d = nc.dram_tensor("bias2", [P, H], F16, kind="ExternalInput").ap()
    scale_d = nc.dram_tensor("scale", [P, MT], F32, kind="ExternalInput").ap()
    out_d = nc.dram_tensor("out", [MT, P, H], F16, kind="ExternalOutput").ap()
    if B:
        xtb_d = [nc.dram_tensor(f"xtb{gi}", [P, len(g) * B], DT,
                                kind="ExternalInput").ap()
                 for gi, g in enumerate(HGROUPS)]

    # k -> (group index, index within group)
    hgi = {k: (gi, i) for gi, g in enumerate(HGROUPS) for i, k in enumerate(g)}

    m_chunks = [list(range(s, min(s + CH, MT))) for s in range(0, MT, CH)]

    with tile.TileContext(nc) as tc:
        with (
            tc.tile_pool(name="ins", bufs=1) as ins,
            tc.tile_pool(name="psum", bufs=1, space="PSUM") as psum_pool,
            tc.tile_pool(name="outp", bufs=2) as outp,
        ):
            blk_sb = [ins.tile([P, BLKW], DT, name=f"blk{j}")
                      for j in range(len(KPAIRS))]
            wB_sb = [ins.tile([P, len(g) * NFREE], DT, name=f"wB{gi}")
                     for gi, g in enumerate(HGROUPS)]
            w2_sb = [ins.tile([P, len(g) * H], DT, name=f"w2_{gi}")
                     for gi, g in enumerate(HGROUPS)]
            xtB_sb = ([ins.tile([P, len(g) * B], DT, name=f"xtB{gi}")
                       for gi, g in enumerate(HGROUPS)] if B else None)
            bias_sb = ins.tile([P, H], F16, name="bias")
            bias2_sb = ins.tile([P, H], F16, name="bias2")
            scale_sb = ins.tile([P, MT], F32, name="scale")

            def xt_ap(k, m):
                j, kk = divmod(k, 2)
                if m < CH:
                    off = kk * A + m * P
                    return blk_sb[j][:, off:off + P]
                gi, i = hgi[k]
                off = i * B + (m - CH) * P
                return xtB_sb[gi][:, off:off + P]

            def w_ap(k, n, sec):
                if sec:
                    gi, i = hgi[k]
                    off = i * H + n * NFREE
                    return w2_sb[gi][:, off:off + NFREE]
                if n == 0:
                    j, kk = divmod(k, 2)
                    off = 2 * A + kk * NFREE
                    return blk_sb[j][:, off:off + NFREE]
                gi, i = hgi[k]
                return wB_sb[gi][:, i * NFREE:(i + 1) * NFREE]

            # PE warm-up: see (a) above.  Small zero tile so the memset
            # finishes fast; GpSimd reaches its first user instruction
            # soonest after the init barrier, so the memset goes there.
            wz = ins.tile([P, NFREE], DT, name="wz")
            nc.gpsimd.memset(wz[:], 0)
            warm_ps = psum_pool.tile([P, NFREE], F32, name="ps3_1")
            for _ in range(WARM):
                nc.tensor.matmul(warm_ps[:, :NFREE - P], wz[:, :P],
                                 wz[:, P:], start=True, stop=True)

            # Input stream: k-pair blocks alternating between the two
            # HWDGE rings in consumption order (see (b) above) — the
            # whole n0 pass's data is on the wire within the first two
            # ring slots of each ring.
            S, Q = nc.sync, nc.scalar
            blk_dmas = [(S if j % 2 == 0 else Q).dma_start(blk_sb[j][:],
                                                           blk_d[j])
                        for j in range(len(KPAIRS))]
            # The n0-half of the bias rides the scalar ring (slack slot
            # before wb4567) instead of SWDGE: a SWDGE transfer in the
            # 12-15 us window takes a full SDMA round-robin share and
            # stretches the k45/k67 block wires past their consumption.
            Q.dma_start(bias_sb[:, :NFREE], biasa_d[:])
            wb_dmas = [S.dma_start(wB_sb[0][:], wb_d[0]),
                       Q.dma_start(wB_sb[1][:], wb_d[1])]
            xtb_dmas = []
            if B:
                xtb_dmas = [S.dma_start(xtB_sb[0][:], xtb_d[0]),
                            Q.dma_start(xtB_sb[1][:], xtb_d[1])]

            # SWDGE side stream, gated so each piece only competes with
            # ring transfers that have slack (see (c) above).  The tiny
            # scale tile is negligible; the fat bias halves wait for the
            # early stream to clear.
            for tgt, src, gate in (
                (scale_sb[:], scale_d[:], blk_dmas[1]),
                (bias_sb[:, NFREE:], biasb_d[:], wb_dmas[0]),
                (bias2_sb[:], bias2_d[:], xtb_dmas[0] if B else wb_dmas[1]),
            ):
                dma = nc.gpsimd.dma_start(tgt, src)
                tile.add_dep_helper(dma.ins, gate.ins,
                                    reason="SWDGE behind ring slack")

            # Half-chunk passes: (4m x 1n) accumulated over k, evicted
            # while the sibling n-half accumulates on the other banks.
            pass_idx = 0
            for chunk in m_chunks:
                for n in range(NT):
                    nsl = slice(n * NFREE, (n + 1) * NFREE)
                    ps = {}
                    for m in chunk:
                        ps[m] = psum_pool.tile([P, NFREE], F32,
                                               name=f"ps{m % CH}_{n}")
                    for k in range(KT):
                        for m in chunk:
                            nc.tensor.matmul(
                                ps[m][:],
                                xt_ap(k, m),
                                w_ap(k, n, sec=(m == MT - 1)),
                                start=(k == 0), stop=(k == KT - 1),
                            )
                    last_pass = (chunk is m_chunks[-1]) and n == NT - 1
                    for mi, m in enumerate(chunk):
                        bsb = bias2_sb if m == MT - 1 else bias_sb
                        t = outp.tile([P, NFREE], F16, name=f"osb{m % CH}_{n}")
                        if last_pass and mi == len(chunk) - 1:
                            # The very last eviction + output gate the
                            # kernel tail: split into halves so the
                            # first out-DMA's descriptor generation and
                            # wire time overlap the second eviction, on
                            # both engines in parallel.
                            hw = NFREE // 2
                            for h, veng in enumerate((nc.vector, nc.vector)):
                                hs = slice(h * hw, (h + 1) * hw)
                                ns2 = slice(n * NFREE + h * hw,
                                            n * NFREE + (h + 1) * hw)
                                veng.scalar_tensor_tensor(
                                    t[:, hs], bsb[:, ns2],
                                    scale_sb[:, m:m + 1], ps[m][:, hs],
                                    op0=ALU.mult, op1=ALU.add,
                                )
                                eng = S if h == 0 else Q
                                eng.dma_start(out_d[m][:, ns2], t[:, hs])
                            continue
                        # out = bias * top_p + psum   (single DVE op)
                        nc.vector.scalar_tensor_tensor(
                            t[:], bsb[:, nsl],
                            scale_sb[:, m:m + 1], ps[m][:],
                            op0=ALU.mult, op1=ALU.add,
                        )
                        eng = S if (pass_idx + mi) % 2 == 0 else Q
                        eng.dma_start(out_d[m][:, nsl], t[:])
                    if pass_idx == 0:
                        # Secondary (overflow) weights: consumed only by
                        # the last m-tile (~75% through the stream) —
                        # queue them behind the first pass's outputs so
                        # they never compete with latency-critical data.
                        S.dma_start(w2_sb[0][:], w2_d[0])
                        Q.dma_start(w2_sb[1][:], w2_d[1])
                    pass_idx += 1

    nc.compile()
    return nc


def _plan(counts):
    """Pick MT and the overflow assignment.

    Returns (MT, prim, ext, free) where each core's secondary (overflow)
    m-tile holds up to 128 tokens: its own expert's overflow beyond
    (MT-1)*128, or one foreign chunk of an overloaded expert.
    Feasibility: every expert's tokens beyond MT*128 must fit in
    128-token chunks on cores whose own expert fits in (MT-1)*128.
    """
    mt_hi = max(1, int(-(-counts.max() // P)))          # plain expert-parallel
    mt_lo = max(1, int(-(-(counts.sum() // E) // P)))
    for MT in range(mt_lo, mt_hi + 1):
        prim = (MT - 1) * P
        ext = [max(0, int(c) - MT * P) for c in counts]
        slots_needed = sum(-(-x // P) for x in ext)
        free = [e for e in range(E) if counts[e] <= prim]
        if slots_needed <= len(free):
            return MT, prim, ext, free
    MT = mt_hi
    prim = (MT - 1) * P
    return MT, prim, [0] * E, []


def kernel(input, gate, W, b):
    from concourse import bass_utils

    input = np.ascontiguousarray(input, dtype=np.float32)
    gate = np.ascontiguousarray(gate, dtype=np.float32)
    W = np.ascontiguousarray(W, dtype=np.float32)
    b = np.ascontiguousarray(b, dtype=np.float32)

    # ---- router (host): top-1 expert + its softmax probability ----
    g = gate.astype(np.float64)
    gm = g.max(axis=1, keepdims=True)
    top_p = (1.0 / np.exp(g - gm).sum(axis=1)).astype(np.float32)
    e_t = np.argmax(gate, axis=1)

    counts = np.bincount(e_t, minlength=E)
    order = np.argsort(e_t, kind="stable")
    starts = np.zeros(E + 1, dtype=np.int64)
    np.cumsum(counts, out=starts[1:])
    ids_of = [order[starts[e]:starts[e + 1]] for e in range(E)]

    MT, prim, ext, free = _plan(counts)
    C = MT * P
    A = min(CH, MT) * P
    B = C - A

    # Per-core token layout: primary expert tokens in cols [0, prim) and
    # own-overflow (up to 128) in the overflow slot; foreign chunks of
    # overloaded experts go to free cores' overflow slots.
    core_prim_ids = []      # ids in the primary region
    core_sec_ids = []       # ids in the overflow m-tile
    core_sec_expert = []
    for e in range(E):
        ids = ids_of[e]
        n_own_prim = min(len(ids), prim)
        n_own_sec = min(P, max(0, len(ids) - prim))
        core_prim_ids.append(ids[:n_own_prim])
        core_sec_ids.append(ids[n_own_prim:n_own_prim + n_own_sec])
        core_sec_expert.append(e)
    # distribute external overflow chunks to free cores
    free_iter = iter(free)
    for e in range(E):
        leftover = ids_of[e][prim + P:] if len(ids_of[e]) > prim + P else []
        o = 0
        while o < len(leftover):
            host = next(free_iter)
            chunk = leftover[o:o + P]
            core_sec_ids[host] = chunk
            core_sec_expert[host] = e
            o += P

    W16 = W.astype(np.float16)
    b16 = b.astype(np.float16)

    if MT not in _BUILD_CACHE:
        _BUILD_CACHE[MT] = _build(MT)
    nc = _BUILD_CACHE[MT]

    in_maps = []
    for e in range(E):
        pi, si, se = core_prim_ids[e], core_sec_ids[e], core_sec_expert[e]
        n_p, n_s = len(pi), len(si)

        xt = np.zeros((KT, P, C), dtype=np.float16)
        xtf = xt.reshape(H, C)
        if n_p:
            xtf[:, :n_p] = (input[pi].T * top_p[pi][None, :]).astype(np.float16)
        if n_s:
            xtf[:, prim:prim + n_s] = (input[si].T * top_p[si][None, :]).astype(np.float16)

        scale = np.zeros((MT, P), dtype=np.float32)
        sf = scale.reshape(C)
        sf[:n_p] = top_p[pi]
        sf[prim:prim + n_s] = top_p[si]
        scale = np.ascontiguousarray(scale.T)

        we = W16[e].reshape(KT, P, H)
        w2 = W16[se].reshape(KT, P, H)

        def cat(src, g, sl):  # column-concat of k-slices
            return np.ascontiguousarray(
                np.concatenate([src[k][:, sl] for k in g], axis=1))

        bfull = np.ascontiguousarray(np.broadcast_to(b16[e], (P, H)))
        im = {
            "biasa": np.ascontiguousarray(bfull[:, :NFREE]),
            "biasb": np.ascontiguousarray(bfull[:, NFREE:]),
            "bias2": np.ascontiguousarray(np.broadcast_to(b16[se], (P, H))),
            "scale": scale,
        }
        for j, (ka, kb) in enumerate(KPAIRS):
            im[f"blk{j}"] = np.ascontiguousarray(np.concatenate(
                [xt[ka][:, :A], xt[kb][:, :A],
                 we[ka][:, :NFREE], we[kb][:, :NFREE]], axis=1))
        for gi, gk in enumerate(HGROUPS):
            im[f"wb{gi}"] = cat(we, gk, slice(NFREE, H))
            im[f"w2_{gi}"] = cat(w2, gk, slice(0, H))
            if B:
                im[f"xtb{gi}"] = cat(xt, gk, slice(A, C))
        in_maps.append(im)

    res = bass_utils.run_bass_kernel_spmd(nc, in_maps,
                                          core_ids=list(range(N_CORES)))

    out = np.empty((T, H), dtype=np.float32)
    for e in range(E):
        r = res.results[e]["out"].reshape(C, H)
        pi, si = core_prim_ids[e], core_sec_ids[e]
        if len(pi):
            out[pi] = r[:len(pi)].astype(np.float32)
        if len(si):
            out[si] = r[prim:prim + len(si)].astype(np.float32)
    return out


# revision 32
# speedup vs baseline: 1.0298x; 1.0298x over previous
"""Mixture-of-Experts (top-1 routing) Trainium2 kernel.

Strategy (expert-parallel with one overflow slot, per sharding hint):
 - Router (softmax / argmax / top-prob) evaluated on host — 8192x8, i.e.
   0.002% of the FLOPs; its cost is dispatch bookkeeping.
 - Core e owns expert e.  The first MT-1 m-tiles of a core hold tokens of
   its primary expert; the last m-tile is an overflow slot (own-expert
   overflow, or up to 128 tokens of one overloaded foreign expert, using
   the core's secondary weight tensor).
 - Each core runs a dense [C,1024] @ [1024,1024] GEMM on the TensorEngine
   with fp16 operands and fp32 PSUM accumulation (~4.5e-4 max rel err
   end-to-end).  PSUM eviction fuses the bias in a single DVE op per
   half-tile: out = (bias * top_p) + psum.

Pipeline layout (v7), engineered around three measured hardware facts:
 (a) The PE HAM clock gate opens (1.2 -> 2.4 GHz) at the first
     fully-busy free-running ~3.4 us activity window, and ANY PE idle
     gap before then (or a >3.4 us gap later) costs a half-clock
     window.  Warm-up matmuls bridge from the earliest possible
     instruction (~7.5 us) to the first real k-tile, and the input
     stream is paced so the PE never starves.
 (b) Each HWDGE ring drains its transfers near-serially at a rate that
     grows with the per-partition row size, with ~1.5 us first-transfer
     arm latency and ~0.5 us completion fan-in.  So: few transfers, in
     consumption order, host-packed contiguous; k0 and k1 ship solo for
     the earliest possible stream start, later k's in pairs.
 (c) The 16 SDMA engines round-robin all active queues, so a transfer's
     wire time stretches with whatever else is in flight: the SWDGE
     (GpSimd) side stream of scale/bias tiles is split and gated so
     each piece moves only after the ring transfers it would compete
     with have completed, and the bulky overflow weights (needed at
     ~75% of the stream) are queued behind the first output tiles.
 - Matmuls run in half-chunk passes: (4 m-tiles x 1 n-half) accumulated
   over all k.  A pass's 4 PSUM banks are evicted while the next pass
   runs on the other 4 banks, pipelining evictions and output DMAs
   behind the TensorEngine instead of bunching them at the tail.
 - Host scatters the compact per-core outputs back to token order
   (the "second all-to-all" / unshard step).
"""

import numpy as np

T, H, E = 8192, 1024, 8
N_CORES = 8
P = 128
KT = H // P          # 8 contraction tiles
NFREE = 512          # matmul moving free dim (one PSUM bank of fp32)
NT = H // NFREE      # 2 output column tiles
CH = 4               # m-tiles per half-chunk (4m x 1n = 4 PSUM banks)

KPAIRS = [[0, 1], [2, 3], [4, 5], [6, 7]]      # k-pair block transfers
HGROUPS = [[0, 1, 2, 3], [4, 5, 6, 7]]         # bulk 4-k groups
WARM = 12            # warm-up matmuls (384-col moving, ~320 ns cold each)

_BUILD_CACHE = {}


def _build(MT):
    """Build the SPMD Bass module for MT m-tiles per core (C = MT*128)."""
    import concourse.mybir as mybir
    import concourse.tile as tile
    from concourse import bacc

    C = MT * P
    A = min(CH, MT) * P  # xt columns consumed by the first chunk
    B = C - A            # remaining xt columns (chunks 1..)
    DT = mybir.dt.float16    # half-precision I/O, full-rate matmul
    F32 = mybir.dt.float32
    F16 = mybir.dt.float16
    ALU = mybir.AluOpType

    nc = bacc.Bacc("TRN2", target_bir_lowering=False, debug=False,
                   num_devices=N_CORES)

    # blk j = [xt_2j[:, :A] | xt_2j+1[:, :A] | w_2j[:, :NF] | w_2j+1[:, :NF]]
    BLKW = 2 * A + 2 * NFREE
    blk_d = [nc.dram_tensor(f"blk{j}", [P, BLKW], DT,
                            kind="ExternalInput").ap()
             for j in range(len(KPAIRS))]
    wb_d = [nc.dram_tensor(f"wb{gi}", [P, len(g) * NFREE], DT,
                           kind="ExternalInput").ap()
            for gi, g in enumerate(HGROUPS)]
    w2_d = [nc.dram_tensor(f"w2_{gi}", [P, len(g) * H], DT,
                           kind="ExternalInput").ap()
            for gi, g in enumerate(HGROUPS)]
    biasa_d = nc.dram_tensor("biasa", [P, NFREE], F16, kind="ExternalInput").ap()
    biasb_d = nc.dram_tensor("biasb", [P, NFREE], F16, kind="ExternalInput").ap()
    bias2_d = nc.dram_tensor("bias2", [P, H], F16, kind="ExternalInput").ap()
    scale_d = nc.dram_tensor("scale", [P, MT], F32, kind="ExternalInput").ap()
    out_d = nc.dram_tensor("out", [MT, P, H], F16, kind="ExternalOutput").ap()
    if B:
        xtb_d = [nc.dram_tensor(f"xtb{gi}", [P, len(g) * B], DT,
                                kind="ExternalInput").ap()
                 for gi, g in enumerate(HGROUPS)]

    # k -> (group index, index within group)
    hgi = {k: (gi, i) for gi, g in enumerate(HGROUPS) for i, k in enumerate(g)}

    m_chunks = [list(range(s, min(s + CH, MT))) for s in range(0, MT, CH)]

    with tile.TileContext(nc) as tc:
        with (
            tc.tile_pool(name="ins", bufs=1) as ins,
            tc.tile_pool(name="psum", bufs=1, space="PSUM") as psum_pool,
            tc.tile_pool(name="outp", bufs=2) as outp,
        ):
            blk_sb = [ins.tile([P, BLKW], DT, name=f"blk{j}")
                      for j in range(len(KPAIRS))]
            wB_sb = [ins.tile([P, len(g) * NFREE], DT, name=f"wB{gi}")
                     for gi, g in enumerate(HGROUPS)]
            w2_sb = [ins.tile([P, len(g) * H], DT, name=f"w2_{gi}")
                     for gi, g in enumerate(HGROUPS)]
            xtB_sb = ([ins.tile([P, len(g) * B], DT, name=f"xtB{gi}")
                       for gi, g in enumerate(HGROUPS)] if B else None)
            bias_sb = ins.tile([P, H], F16, name="bias")
            bias2_sb = ins.tile([P, H], F16, name="bias2")
            scale_sb = ins.tile([P, MT], F32, name="scale")

            def xt_ap(k, m):
                j, kk = divmod(k, 2)
                if m < CH:
                    off = kk * A + m * P
                    return blk_sb[j][:, off:off + P]
                gi, i = hgi[k]
                off = i * B + (m - CH) * P
                return xtB_sb[gi][:, off:off + P]

            def w_ap(k, n, sec):
                if sec:
                    gi, i = hgi[k]
                    off = i * H + n * NFREE
                    return w2_sb[gi][:, off:off + NFREE]
                if n == 0:
                    j, kk = divmod(k, 2)
                    off = 2 * A + kk * NFREE
                    return blk_sb[j][:, off:off + NFREE]
                gi, i = hgi[k]
                return wB_sb[gi][:, i * NFREE:(i + 1) * NFREE]

            # PE warm-up: see (a) above.  Small zero tile so the memset
            # finishes fast; GpSimd reaches its first user instruction
            # soonest after the init barrier, so the memset goes there.
            wz = ins.tile([P, NFREE], DT, name="wz")
            nc.gpsimd.memset(wz[:], 0)
            warm_ps = psum_pool.tile([P, NFREE], F32, name="ps3_1")
            for _ in range(WARM):
                nc.tensor.matmul(warm_ps[:, :NFREE - P], wz[:, :P],
                                 wz[:, P:], start=True, stop=True)

            # Input stream: k-pair blocks alternating between the two
            # HWDGE rings in consumption order (see (b) above) — the
            # whole n0 pass's data is on the wire within the first two
            # ring slots of each ring.
            S, Q = nc.sync, nc.scalar
            blk_dmas = [(S if j % 2 == 0 else Q).dma_start(blk_sb[j][:],
                                                           blk_d[j])
                        for j in range(len(KPAIRS))]
            wb_dmas = [S.dma_start(wB_sb[0][:], wb_d[0]),
                       Q.dma_start(wB_sb[1][:], wb_d[1])]
            xtb_dmas = []
            if B:
                xtb_dmas = [S.dma_start(xtB_sb[0][:], xtb_d[0]),
                            Q.dma_start(xtB_sb[1][:], xtb_d[1])]

            # SWDGE side stream, gated so each piece only competes with
            # ring transfers that have slack (see (c) above).
            for tgt, src, gate in (
                (scale_sb[:], scale_d[:], blk_dmas[1]),
                (bias_sb[:, :NFREE], biasa_d[:], blk_dmas[1]),
                (bias_sb[:, NFREE:], biasb_d[:], wb_dmas[0]),
                (bias2_sb[:], bias2_d[:], xtb_dmas[0] if B else wb_dmas[1]),
            ):
                dma = nc.gpsimd.dma_start(tgt, src)
                tile.add_dep_helper(dma.ins, gate.ins,
                                    reason="SWDGE behind ring slack")

            # Half-chunk passes: (4m x 1n) accumulated over k, evicted
            # while the sibling n-half accumulates on the other banks.
            pass_idx = 0
            for chunk in m_chunks:
                for n in range(NT):
                    nsl = slice(n * NFREE, (n + 1) * NFREE)
                    ps = {}
                    for m in chunk:
                        ps[m] = psum_pool.tile([P, NFREE], F32,
                                               name=f"ps{m % CH}_{n}")
                    for k in range(KT):
                        for m in chunk:
                            nc.tensor.matmul(
                                ps[m][:],
                                xt_ap(k, m),
                                w_ap(k, n, sec=(m == MT - 1)),
                                start=(k == 0), stop=(k == KT - 1),
                            )
                    last_pass = (chunk is m_chunks[-1]) and n == NT - 1
                    for mi, m in enumerate(chunk):
                        bsb = bias2_sb if m == MT - 1 else bias_sb
                        t = outp.tile([P, NFREE], F16, name=f"osb{m % CH}_{n}")
                        if last_pass and mi == len(chunk) - 1:
                            # The very last eviction + output gate the
                            # kernel tail: split into halves so the
                            # first out-DMA's descriptor generation and
                            # wire time overlap the second eviction, on
                            # both engines in parallel.
                            hw = NFREE // 2
                            for h, veng in enumerate((nc.vector, nc.vector)):
                                hs = slice(h * hw, (h + 1) * hw)
                                ns2 = slice(n * NFREE + h * hw,
                                            n * NFREE + (h + 1) * hw)
                                veng.scalar_tensor_tensor(
                                    t[:, hs], bsb[:, ns2],
                                    scale_sb[:, m:m + 1], ps[m][:, hs],
                                    op0=ALU.mult, op1=ALU.add,
                                )
                                eng = S if h == 0 else Q
                                eng.dma_start(out_d[m][:, ns2], t[:, hs])
                            continue
                        # out = bias * top_p + psum   (single DVE op)
                        nc.vector.scalar_tensor_tensor(
                            t[:], bsb[:, nsl],
                            scale_sb[:, m:m + 1], ps[m][:],
                            op0=ALU.mult, op1=ALU.add,
                        )
                        eng = S if (pass_idx + mi) % 2 == 0 else Q
                        eng.dma_start(out_d[m][:, nsl], t[:])
                    if pass_idx == 0:
                        # Secondary (overflow) weights: consumed only by
                        # the last m-tile (~75% through the stream) —
                        # queue them behind the first pass's outputs so
                        # they never compete with latency-critical data.
                        S.dma_start(w2_sb[0][:], w2_d[0])
                        Q.dma_start(w2_sb[1][:], w2_d[1])
                    pass_idx += 1

    nc.compile()
    return nc


def _plan(counts):
    """Pick MT and the overflow assignment.

    Returns (MT, prim, ext, free) where each core's secondary (overflow)
    m-tile holds up to 128 tokens: its own expert's overflow beyond
    (MT-1)*128, or one foreign chunk of an overloaded expert.
    Feasibility: every expert's tokens beyond MT*128 must fit in
    128-token chunks on cores whose own expert fits in (MT-1)*128.
    """
    mt_hi = max(1, int(-(-counts.max() // P)))          # plain expert-parallel
    mt_lo = max(1, int(-(-(counts.sum() // E) // P)))
    for MT in range(mt_lo, mt_hi + 1):
        prim = (MT - 1) * P
        ext = [max(0, int(c) - MT * P) for c in counts]
        slots_needed = sum(-(-x // P) for x in ext)
        free = [e for e in range(E) if counts[e] <= prim]
        if slots_needed <= len(free):
            return MT, prim, ext, free
    MT = mt_hi
    prim = (MT - 1) * P
    return MT, prim, [0] * E, []


def kernel(input, gate, W, b):
    from concourse import bass_utils

    input = np.ascontiguousarray(input, dtype=np.float32)
    gate = np.ascontiguousarray(gate, dtype=np.float32)
    W = np.ascontiguousarray(W, dtype=np.float32)
    b = np.ascontiguousarray(b, dtype=np.float32)

    # ---- router (host): top-1 expert + its softmax probability ----
    g = gate.astype(np.float64)
    gm = g.max(axis=1, keepdims=True)
    top_p = (1.0 / np.exp(g - gm).sum(axis=1)).astype(np.float32)
    e_t = np.argmax(gate, axis=1)

    counts = np.bincount(e_t, minlength=E)
    order = np.argsort(e_t, kind="stable")
    starts = np.zeros(E + 1, dtype=np.int64)
    np.cumsum(counts, out=starts[1:])
    ids_of = [order[starts[e]:starts[e + 1]] for e in range(E)]

    MT, prim, ext, free = _plan(counts)
    C = MT * P
    A = min(CH, MT) * P
    B = C - A

    # Per-core token layout: primary expert tokens in cols [0, prim) and
    # own-overflow (up to 128) in the overflow slot; foreign chunks of
    # overloaded experts go to free cores' overflow slots.
    core_prim_ids = []      # ids in the primary region
    core_sec_ids = []       # ids in the overflow m-tile
    core_sec_expert = []
    for e in range(E):
        ids = ids_of[e]
        n_own_prim = min(len(ids), prim)
        n_own_sec = min(P, max(0, len(ids) - prim))
        core_prim_ids.append(ids[:n_own_prim])
        core_sec_ids.append(ids[n_own_prim:n_own_prim + n_own_sec])
        core_sec_expert.append(e)
    # distribute external overflow chunks to free cores
    free_iter = iter(free)
    for e in range(E):
        leftover = ids_of[e][prim + P:] if len(ids_of[e]) > prim + P else []
        o = 0
        while o < len(leftover):
            host = next(free_iter)
            chunk = leftover[o:o + P]
            core_sec_ids[host] = chunk
            core_sec_expert[host] = e
            o += P

    W16 = W.astype(np.float16)
    b16 = b.astype(np.float16)

    if MT not in _BUILD_CACHE:
        _BUILD_CACHE[MT] = _build(MT)
    nc = _BUILD_CACHE[MT]

    in_maps = []
    for e in range(E):
        pi, si, se = core_prim_ids[e], core_sec_ids[e], core_sec_expert[e]
        n_p, n_s = len(pi), len(si)

        xt = np.zeros((KT, P, C), dtype=np.float16)
        xtf = xt.reshape(H, C)
        if n_p:
            xtf[:, :n_p] = (input[pi].T * top_p[pi][None, :]).astype(np.float16)
        if n_s:
            xtf[:, prim:prim + n_s] = (input[si].T * top_p[si][None, :]).astype(np.float16)

        scale = np.zeros((MT, P), dtype=np.float32)
        sf = scale.reshape(C)
        sf[:n_p] = top_p[pi]
        sf[prim:prim + n_s] = top_p[si]
        scale = np.ascontiguousarray(scale.T)

        we = W16[e].reshape(KT, P, H)
        w2 = W16[se].reshape(KT, P, H)

        def cat(src, g, sl):  # column-concat of k-slices
            return np.ascontiguousarray(
                np.concatenate([src[k][:, sl] for k in g], axis=1))

        bfull = np.ascontiguousarray(np.broadcast_to(b16[e], (P, H)))
        im = {
            "biasa": np.ascontiguousarray(bfull[:, :NFREE]),
            "biasb": np.ascontiguousarray(bfull[:, NFREE:]),
            "bias2": np.ascontiguousarray(np.broadcast_to(b16[se], (P, H))),
            "scale": scale,
        }
        for j, (ka, kb) in enumerate(KPAIRS):
            im[f"blk{j}"] = np.ascontiguousarray(np.concatenate(
                [xt[ka][:, :A], xt[kb][:, :A],
                 we[ka][:, :NFREE], we[kb][:, :NFREE]], axis=1))
        for gi, gk in enumerate(HGROUPS):
            im[f"wb{gi}"] = cat(we, gk, slice(NFREE, H))
            im[f"w2_{gi}"] = cat(w2, gk, slice(0, H))
            if B:
                im[f"xtb{gi}"] = cat(xt, gk, slice(A, C))
        in_maps.append(im)

    res = bass_utils.run_bass_kernel_spmd(nc, in_maps,
                                          core_ids=list(range(N_CORES)))

    out = np.empty((T, H), dtype=np.float32)
    for e in range(E):
        r = res.results[e]["out"].reshape(C, H)
        pi, si = core_prim_ids[e], core_sec_ids[e]
        if len(pi):
            out[pi] = r[:len(pi)].astype(np.float32)
        if len(si):
            out[si] = r[prim:prim + len(si)].astype(np.float32)
    return out
